# revision 23
# baseline (speedup 1.0000x reference)
"""CTLSTM cell fused kernel for 8 Trainium2 NeuronCores.

Strategy (data-parallel over batch, TRANSPOSED compute):
  - B=16384 rows sharded 2048/core; weights replicated.
  - Compute g.T: gates on SBUF partitions, batch on the free dim.
    Stationary operand = weight tile [K=128, 128 gates]; moving operand =
    xh [K=128, 1024 batch] bf16 (max bf16 moving free dim). Each PSUM tile
    is one gate-tile x batch-half: [128, 1024] fp32 (2 banks).
  - With gates on partitions the bias is per-partition: the ACT engine
    drains PSUM directly with out = act(psum*scale + bias[p]) in ONE op -
    no DVE bias-add drain at all. DVE only runs the elementwise chain.
  - Gate-group exec order [d, z, i, f, ib, fb, o]:
      * d first: softplus(wd) = -ln(sigmoid(-wd)). The Ln burst (one act
        table switch to natural_log and back) slots in right after the
        4 d-tiles of each half, mid-stream where ACT has slack.
      * o last: the tail after the final matmul is just sigmoid(o) +
        h = o*tanh(c) + store.
  - DMA priority: xh half-0 chunks + first weight tiles first (weights
    staged host-side as contiguous 256 KiB tiles in exec order) so the
    PE starts ~14 us in; weight arrival then stays ahead of the PE.
  - Outputs h/o/dr stored bf16 (halved write traffic), c/cb fp32.
    Host transposes back and upcasts.
"""

import numpy as np
import ml_dtypes

import concourse.bacc as bacc
import concourse.bass as bass
import concourse.mybir as mybir
import concourse.tile as tile
from concourse.bass_utils import run_bass_kernel_spmd

NCORES = 8
B = 16384
I = 512
H = 512
NG = 7
G = NG * H          # 3584
K2 = I + H          # 1024
P = 128
BS = B // NCORES    # 2048 batch cols per core
NH = 2              # batch halves of 1024
BN = BS // NH       # 1024
NQ = H // P         # 4 hidden quadrants (128 gate rows each)
NK = K2 // P        # 8 contraction chunks
NGT = G // P        # 28 gate tiles

BF16 = mybir.dt.bfloat16
F32 = mybir.dt.float32
AF = mybir.ActivationFunctionType
NPBF16 = ml_dtypes.bfloat16

# gate-group exec order: d, z, i, f, ib, fb, o
# reference row order is   i, f, z, o, d, ib, fb
SRC = [4, 2, 0, 1, 5, 6, 3]
GD, GZ, GI, GF, GIB, GFB, GO = range(7)

TRACE = False
LAST_RESULTS = None

_nc_cache = None


def _build():
    nc = bacc.Bacc("TRN2", target_bir_lowering=False, debug=False)

    xh_d = nc.dram_tensor("xh", [NH, NK, P, BN], BF16, kind="ExternalInput")
    w_d = nc.dram_tensor("w", [7, P, NQ * NK * P], BF16, kind="ExternalInput")
    ct_d = nc.dram_tensor("ct", [H, BS], BF16, kind="ExternalInput")
    bb_d = nc.dram_tensor("bb", [P, NGT], F32, kind="ExternalInput")

    h_d = nc.dram_tensor("h", [H, BS], BF16, kind="ExternalOutput")
    c_d = nc.dram_tensor("c", [H, BS], F32, kind="ExternalOutput")
    cb_d = nc.dram_tensor("cb", [H, BS], F32, kind="ExternalOutput")
    o_d = nc.dram_tensor("o", [H, BS], BF16, kind="ExternalOutput")
    dr_d = nc.dram_tensor("dr", [H, BS], BF16, kind="ExternalOutput")

    with tile.TileContext(nc) as tc:
        with (
            tc.tile_pool(name="wp", bufs=1) as wp,
            tc.tile_pool(name="xp", bufs=1) as xp,
            tc.tile_pool(name="cp", bufs=1) as cp,
            tc.tile_pool(name="gp", bufs=1) as gp,
            tc.tile_pool(name="dp", bufs=1) as dp,
            tc.tile_pool(name="op", bufs=2) as op_,
            tc.tile_pool(name="pp", bufs=4, space=bass.MemorySpace.PSUM) as pp,
        ):
            # --- input DMA issue order = arrival priority ---
            # weights arrive as one 1 MiB DMA per gate-group (fewer Sync
            # issues -> xh chunk issues are not starved during the ramp);
            # group 0 (d) is split so its first tile lands early.
            xh_sb = {}
            w_sb = [None] * 7

            def load_xh(h, k):
                t = xp.tile([P, BN], BF16, tag=f"xh{h}_{k}")
                nc.sync.dma_start(t[:], xh_d[h, k])
                xh_sb[(h, k)] = t

            def load_w(grp, split=False):
                t = wp.tile([P, NQ * NK * P], BF16, tag=f"w{grp}")
                if split:
                    nc.sync.dma_start(t[:, :NK * P], w_d[grp, :, :NK * P])
                    nc.sync.dma_start(t[:, NK * P:], w_d[grp, :, NK * P:])
                else:
                    nc.sync.dma_start(t[:], w_d[grp])
                w_sb[grp] = t

            def w_ap(gt, k):
                grp, q = divmod(gt, NQ)
                base = q * NK * P + k * P
                return w_sb[grp][:, base:base + P]

            load_xh(0, 0)
            load_w(0, split=True)
            for k in range(1, NK):
                load_xh(0, k)
            load_w(1)
            bb = cp.tile([P, NGT], F32, tag="bb")
            nc.sync.dma_start(bb[:], bb_d[:])
            load_w(2)
            for k in range(NK):
                load_xh(1, k)
            load_w(3)
            load_w(4)
            load_w(5)
            ct_sb = []
            for q in range(NQ):
                t = cp.tile([P, BS], BF16, tag=f"ct{q}")
                nc.sync.dma_start(t[:], ct_d[q * P:(q + 1) * P, :])
                ct_sb.append(t)
            load_w(6)

            # --- main loop: halves x gate-groups x quadrants ---
            for h in range(NH):
                col = slice(h * BN, (h + 1) * BN)
                # one [128, 4096] sigmoid(-wd) supertile per half: the Ln
                # is then a single ACT op, so the scheduler cannot
                # interleave it with sigmoid drains (one table switch
                # each way per half instead of per-quadrant)
                sdt = dp.tile([P, NQ * BN], BF16, tag="sd")
                gz = [None] * NQ
                gi = [None] * NQ
                gf = [None] * NQ
                gib = [None] * NQ
                gfb = [None] * NQ
                th = [None] * NQ

                def mm(gt):
                    # moving free dim caps at 512: two accumulation groups
                    # into the two banks of one [128, 1024] PSUM tile.
                    # k-outer/bank-inner reuses each stationary tile.
                    acc = pp.tile([P, BN], F32, tag="acc")
                    for k in range(NK):
                        for bh in range(2):
                            bsl = slice(bh * 512, (bh + 1) * 512)
                            nc.tensor.matmul(
                                acc[:, bsl], w_ap(gt, k),
                                xh_sb[(h, k)][:, bsl],
                                start=(k == 0), stop=(k == NK - 1),
                            )
                    return acc

                def mm_half(gt, bh):
                    # single-bank PSUM slot: drain of bank 0 depends only
                    # on its own 8 matmuls, overlapping bank 1's group
                    acc = pp.tile([P, 512], F32, tag="acc")
                    bsl = slice(bh * 512, (bh + 1) * 512)
                    for k in range(NK):
                        nc.tensor.matmul(
                            acc[:], w_ap(gt, k),
                            xh_sb[(h, k)][:, bsl],
                            start=(k == 0), stop=(k == NK - 1),
                        )
                    return acc

                for grp in range(7):
                    for q in range(NQ):
                        gt = grp * NQ + q
                        rows = slice(q * P, (q + 1) * P)
                        last = (grp == GO and h == NH - 1 and q == NQ - 1)
                        acc = None if last else mm(gt)
                        bias = bb[:, gt:gt + 1]
                        if grp == GD:
                            # sigmoid(-(psum+b)) ; d-bias staged negated
                            nc.scalar.activation(
                                sdt[:, q * BN:(q + 1) * BN], acc[:],
                                AF.Sigmoid, bias=bias, scale=-1.0)
                        elif grp == GZ:
                            g = gp.tile([P, BN], BF16, tag=f"z{q}")
                            nc.scalar.activation(g[:], acc[:], AF.Tanh,
                                                 bias=bias)
                            gz[q] = g
                        elif last:
                            # final tile: per-bank PSUM slots + drains so
                            # the tail after the very last matmul is one
                            # 512-col sigmoid + mul + store
                            g = gp.tile([P, BN], BF16, tag=f"o{q}")
                            hh = op_.tile([P, BN], BF16, tag="hh")
                            for b2 in range(2):
                                acch = mm_half(gt, b2)
                                ssl = slice(b2 * 512, (b2 + 1) * 512)
                                csl = slice(h * BN + b2 * 512,
                                            h * BN + (b2 + 1) * 512)
                                nc.scalar.activation(g[:, ssl], acch[:],
                                                     AF.Sigmoid, bias=bias)
                                nc.sync.dma_start(o_d[rows, csl], g[:, ssl])
                                nc.vector.tensor_mul(hh[:, ssl], g[:, ssl],
                                                     th[q][:, ssl])
                                nc.sync.dma_start(h_d[rows, csl], hh[:, ssl])
                        else:
                            tagn = ("", "", "i", "f", "ib", "fb", "o")[grp]
                            g = gp.tile([P, BN], BF16, tag=f"{tagn}{q}")
                            nc.scalar.activation(g[:], acc[:], AF.Sigmoid,
                                                 bias=bias)
                            if grp == GI:
                                gi[q] = g
                            elif grp == GF:
                                gf[q] = g
                            elif grp == GIB:
                                gib[q] = g
                            elif grp == GFB:
                                gfb[q] = g
                                if q == NQ - 1:
                                    # chain part A for all quadrants
                                    for qq in range(NQ):
                                        rr = slice(qq * P, (qq + 1) * P)
                                        ctq = ct_sb[qq][:, col]
                                        c = op_.tile([P, BN], F32, tag="c")
                                        tmp = op_.tile([P, BN], F32, tag="tmp")
                                        cb = op_.tile([P, BN], F32, tag="cb")
                                        # all 4 live until chain B: bufs=4
                                        t_ = op_.tile([P, BN], BF16, tag="th",
                                                      bufs=4)
                                        nc.vector.tensor_mul(c[:], gf[qq][:], ctq)
                                        nc.vector.tensor_mul(tmp[:], gi[qq][:], gz[qq][:])
                                        nc.vector.tensor_add(c[:], c[:], tmp[:])
                                        nc.sync.dma_start(c_d[rr, col], c[:])
                                        nc.scalar.activation(t_[:], c[:], AF.Tanh)
                                        th[qq] = t_
                                        nc.vector.tensor_mul(cb[:], gfb[qq][:], ctq)
                                        nc.vector.tensor_mul(tmp[:], gib[qq][:], gz[qq][:])
                                        nc.vector.tensor_add(cb[:], cb[:], tmp[:])
                                        nc.sync.dma_start(cb_d[rr, col], cb[:])
                            else:  # GO: chain part B
                                nc.sync.dma_start(o_d[rows, col], g[:])
                                hh = op_.tile([P, BN], BF16, tag="hh")
                                nc.vector.tensor_mul(hh[:], g[:], th[q][:])
                                nc.sync.dma_start(h_d[rows, col], hh[:])
                    if grp == GD:
                        # softplus for this half: -ln(sigmoid(-wd)),
                        # single Ln + negate over the supertile
                        nc.scalar.activation(sdt[:], sdt[:], AF.Ln)
                        nc.vector.tensor_scalar_mul(sdt[:], sdt[:], -1.0)
                        for q in range(NQ):
                            rows = slice(q * P, (q + 1) * P)
                            nc.sync.dma_start(dr_d[rows, col],
                                              sdt[:, q * BN:(q + 1) * BN])

    nc.compile()
    return nc


def kernel(x, ht, ct, Wx, bx, Wh, bh):
    global _nc_cache, LAST_RESULTS
    if _nc_cache is None:
        _nc_cache = _build()
    nc = _nc_cache

    x = np.ascontiguousarray(x, dtype=np.float32)
    ht = np.ascontiguousarray(ht, dtype=np.float32)
    ct = np.ascontiguousarray(ct, dtype=np.float32)

    # weights: [K2, G] in exec gate order, tiled [28][128][8*128]
    WxT = np.asarray(Wx, dtype=np.float32).T   # [512, 3584]
    WhT = np.asarray(Wh, dtype=np.float32).T
    bsum = np.asarray(bx, dtype=np.float32) + np.asarray(bh, dtype=np.float32)
    w2 = np.empty((K2, G), dtype=NPBF16)
    bbp = np.empty(G, dtype=np.float32)
    for n, old in enumerate(SRC):
        dsl = slice(n * H, (n + 1) * H)
        ssl = slice(old * H, (old + 1) * H)
        w2[:I, dsl] = WxT[:, ssl].astype(NPBF16)
        w2[I:, dsl] = WhT[:, ssl].astype(NPBF16)
        bbp[dsl] = bsum[ssl]
    bbp[0:H] = -bbp[0:H]           # d-gate bias negated (scale=-1 trick)
    # w_stage[grp, p, q*1024 + k*128 + g] = w2[k*128+p, (grp*4+q)*128+g]
    w_stage = np.ascontiguousarray(
        w2.reshape(NK, P, 7, NQ, P).transpose(2, 1, 3, 0, 4)
        .reshape(7, P, NQ * NK * P)
    )
    bbT = np.ascontiguousarray(bbp.reshape(NGT, P).T)   # [128, 28]

    in_maps = []
    for cidx in range(NCORES):
        sl = slice(cidx * BS, (cidx + 1) * BS)
        xh_full = np.empty((K2, BS), dtype=NPBF16)
        xh_full[:I, :] = x[sl].T.astype(NPBF16)
        xh_full[I:, :] = ht[sl].T.astype(NPBF16)
        # [2, 8, 128, 1024] halves-major
        xh_stage = np.ascontiguousarray(
            xh_full.reshape(NK, P, NH, BN).transpose(2, 0, 1, 3)
        )
        ctT = np.ascontiguousarray(ct[sl].T.astype(NPBF16))
        in_maps.append({
            "xh": xh_stage,
            "w": w_stage,
            "ct": ctT,
            "bb": bbT,
        })

    res = run_bass_kernel_spmd(nc, in_maps, core_ids=list(range(NCORES)),
                               trace=TRACE)
    LAST_RESULTS = res

    outs = {}
    for name in ("h", "c", "cb", "o", "dr"):
        full = np.concatenate(
            [res.results[cidx][name] for cidx in range(NCORES)], axis=1
        )
        outs[name] = np.ascontiguousarray(full.T.astype(np.float32))
    return outs["h"], outs["c"], outs["cb"], outs["o"], outs["dr"]


# revision 25
# speedup vs baseline: 1.0058x; 1.0058x over previous
"""CTLSTM cell fused kernel for 8 Trainium2 NeuronCores.

Strategy (data-parallel over batch, TRANSPOSED compute):
  - B=16384 rows sharded 2048/core; weights replicated.
  - Compute g.T: gates on SBUF partitions, batch on the free dim.
    Stationary operand = weight tile [K=128, 128 gates]; moving operand =
    xh [K=128, 1024 batch] bf16 (max bf16 moving free dim). Each PSUM tile
    is one gate-tile x batch-half: [128, 1024] fp32 (2 banks).
  - With gates on partitions the bias is per-partition: the ACT engine
    drains PSUM directly with out = act(psum*scale + bias[p]) in ONE op -
    no DVE bias-add drain at all. DVE only runs the elementwise chain.
  - Gate-group exec order [d, z, i, f, ib, fb, o]:
      * d first: softplus(wd) = -ln(sigmoid(-wd)). The Ln burst (one act
        table switch to natural_log and back) slots in right after the
        4 d-tiles of each half, mid-stream where ACT has slack.
      * o last: the tail after the final matmul is just sigmoid(o) +
        h = o*tanh(c) + store.
  - DMA priority: xh half-0 chunks + first weight tiles first (weights
    staged host-side as contiguous 256 KiB tiles in exec order) so the
    PE starts ~14 us in; weight arrival then stays ahead of the PE.
  - Outputs h/o/dr stored bf16 (halved write traffic), c/cb fp32.
    Host transposes back and upcasts.
"""

import numpy as np
import ml_dtypes

import concourse.bacc as bacc
import concourse.bass as bass
import concourse.mybir as mybir
import concourse.tile as tile
from concourse.bass_utils import run_bass_kernel_spmd

NCORES = 8
B = 16384
I = 512
H = 512
NG = 7
G = NG * H          # 3584
K2 = I + H          # 1024
P = 128
BS = B // NCORES    # 2048 batch cols per core
NH = 2              # batch halves of 1024
BN = BS // NH       # 1024
NQ = H // P         # 4 hidden quadrants (128 gate rows each)
NK = K2 // P        # 8 contraction chunks
NGT = G // P        # 28 gate tiles

BF16 = mybir.dt.bfloat16
F32 = mybir.dt.float32
AF = mybir.ActivationFunctionType
NPBF16 = ml_dtypes.bfloat16

# gate-group exec order: d, z, i, f, ib, fb, o
# reference row order is   i, f, z, o, d, ib, fb
SRC = [4, 2, 0, 1, 5, 6, 3]
GD, GZ, GI, GF, GIB, GFB, GO = range(7)

TRACE = False
LAST_RESULTS = None

_nc_cache = None


def _build():
    nc = bacc.Bacc("TRN2", target_bir_lowering=False, debug=False)

    xh_d = nc.dram_tensor("xh", [NH, NK, P, BN], BF16, kind="ExternalInput")
    w_d = nc.dram_tensor("w", [7, P, NQ * NK * P], BF16, kind="ExternalInput")
    ct_d = nc.dram_tensor("ct", [H, BS], BF16, kind="ExternalInput")
    bb_d = nc.dram_tensor("bb", [P, NGT], F32, kind="ExternalInput")

    h_d = nc.dram_tensor("h", [H, BS], BF16, kind="ExternalOutput")
    c_d = nc.dram_tensor("c", [H, BS], F32, kind="ExternalOutput")
    cb_d = nc.dram_tensor("cb", [H, BS], F32, kind="ExternalOutput")
    o_d = nc.dram_tensor("o", [H, BS], BF16, kind="ExternalOutput")
    dr_d = nc.dram_tensor("dr", [H, BS], BF16, kind="ExternalOutput")

    with tile.TileContext(nc) as tc:
        with (
            tc.tile_pool(name="wp", bufs=1) as wp,
            tc.tile_pool(name="xp", bufs=1) as xp,
            tc.tile_pool(name="cp", bufs=1) as cp,
            tc.tile_pool(name="gp", bufs=1) as gp,
            tc.tile_pool(name="dp", bufs=1) as dp,
            tc.tile_pool(name="op", bufs=2) as op_,
            tc.tile_pool(name="pp", bufs=4, space=bass.MemorySpace.PSUM) as pp,
        ):
            # --- input DMA issue order = arrival priority ---
            # weights arrive as one 1 MiB DMA per gate-group (fewer Sync
            # issues -> xh chunk issues are not starved during the ramp);
            # group 0 (d) is split so its first tile lands early.
            xh_sb = {}
            w_sb = [None] * 7
            w0a = None   # d-q0 weights: own tile -> own dep, earliest MM

            def load_xh(h, k):
                t = xp.tile([P, BN], BF16, tag=f"xh{h}_{k}")
                nc.sync.dma_start(t[:], xh_d[h, k])
                xh_sb[(h, k)] = t

            def load_w(grp):
                t = wp.tile([P, NQ * NK * P], BF16, tag=f"w{grp}")
                nc.sync.dma_start(t[:], w_d[grp])
                w_sb[grp] = t

            def w_ap(gt, k):
                grp, q = divmod(gt, NQ)
                if grp == 0:
                    if q == 0:
                        return w0a[:, k * P:(k + 1) * P]
                    base = (q - 1) * NK * P + k * P
                    return w_sb[0][:, base:base + P]
                base = q * NK * P + k * P
                return w_sb[grp][:, base:base + P]

            load_xh(0, 0)
            w0a = wp.tile([P, NK * P], BF16, tag="w0a")
            nc.sync.dma_start(w0a[:], w_d[0, :, :NK * P])
            load_xh(0, 1)
            w0b = wp.tile([P, 3 * NK * P], BF16, tag="w0b")
            nc.sync.dma_start(w0b[:], w_d[0, :, NK * P:])
            w_sb[0] = w0b
            load_xh(0, 2)
            load_xh(0, 3)
            bb = cp.tile([P, NGT], F32, tag="bb")
            nc.sync.dma_start(bb[:], bb_d[:])
            for k in range(4, NK):
                load_xh(0, k)
            load_w(1)
            load_w(2)
            for k in range(NK):
                load_xh(1, k)
            load_w(3)
            load_w(4)
            load_w(5)
            ct_sb = []
            for q in range(NQ):
                t = cp.tile([P, BS], BF16, tag=f"ct{q}")
                nc.sync.dma_start(t[:], ct_d[q * P:(q + 1) * P, :])
                ct_sb.append(t)
            load_w(6)

            # --- main loop: halves x gate-groups x quadrants ---
            for h in range(NH):
                col = slice(h * BN, (h + 1) * BN)
                # one [128, 4096] sigmoid(-wd) supertile per half: the Ln
                # is then a single ACT op, so the scheduler cannot
                # interleave it with sigmoid drains (one table switch
                # each way per half instead of per-quadrant)
                sdt = dp.tile([P, NQ * BN], BF16, tag="sd")
                gz = [None] * NQ
                gi = [None] * NQ
                gf = [None] * NQ
                gib = [None] * NQ
                gfb = [None] * NQ
                th = [None] * NQ

                def mm(gt):
                    # moving free dim caps at 512: two accumulation groups
                    # into the two banks of one [128, 1024] PSUM tile.
                    # k-outer/bank-inner reuses each stationary tile.
                    acc = pp.tile([P, BN], F32, tag="acc")
                    for k in range(NK):
                        for bh in range(2):
                            bsl = slice(bh * 512, (bh + 1) * 512)
                            nc.tensor.matmul(
                                acc[:, bsl], w_ap(gt, k),
                                xh_sb[(h, k)][:, bsl],
                                start=(k == 0), stop=(k == NK - 1),
                            )
                    return acc

                def mm_half(gt, bh):
                    # single-bank PSUM slot: drain of bank 0 depends only
                    # on its own 8 matmuls, overlapping bank 1's group
                    acc = pp.tile([P, 512], F32, tag="acc")
                    bsl = slice(bh * 512, (bh + 1) * 512)
                    for k in range(NK):
                        nc.tensor.matmul(
                            acc[:], w_ap(gt, k),
                            xh_sb[(h, k)][:, bsl],
                            start=(k == 0), stop=(k == NK - 1),
                        )
                    return acc

                for grp in range(7):
                    for q in range(NQ):
                        gt = grp * NQ + q
                        rows = slice(q * P, (q + 1) * P)
                        last = (grp == GO and h == NH - 1 and q == NQ - 1)
                        acc = None if last else mm(gt)
                        bias = bb[:, gt:gt + 1]
                        if grp == GD:
                            # sigmoid(-(psum+b)) ; d-bias staged negated
                            nc.scalar.activation(
                                sdt[:, q * BN:(q + 1) * BN], acc[:],
                                AF.Sigmoid, bias=bias, scale=-1.0)
                        elif grp == GZ:
                            g = gp.tile([P, BN], BF16, tag=f"z{q}")
                            nc.scalar.activation(g[:], acc[:], AF.Tanh,
                                                 bias=bias)
                            gz[q] = g
                        elif last:
                            # final tile: per-bank PSUM slots + drains so
                            # the tail after the very last matmul is one
                            # 512-col sigmoid + mul + store
                            g = gp.tile([P, BN], BF16, tag=f"o{q}")
                            hh = op_.tile([P, BN], BF16, tag="hh")
                            for b2 in range(2):
                                acch = mm_half(gt, b2)
                                ssl = slice(b2 * 512, (b2 + 1) * 512)
                                csl = slice(h * BN + b2 * 512,
                                            h * BN + (b2 + 1) * 512)
                                nc.scalar.activation(g[:, ssl], acch[:],
                                                     AF.Sigmoid, bias=bias)
                                nc.sync.dma_start(o_d[rows, csl], g[:, ssl])
                                nc.vector.tensor_mul(hh[:, ssl], g[:, ssl],
                                                     th[q][:, ssl])
                                nc.sync.dma_start(h_d[rows, csl], hh[:, ssl])
                        else:
                            tagn = ("", "", "i", "f", "ib", "fb", "o")[grp]
                            g = gp.tile([P, BN], BF16, tag=f"{tagn}{q}")
                            nc.scalar.activation(g[:], acc[:], AF.Sigmoid,
                                                 bias=bias)
                            if grp == GI:
                                gi[q] = g
                            elif grp == GF:
                                gf[q] = g
                            elif grp == GIB:
                                gib[q] = g
                            elif grp == GFB:
                                gfb[q] = g
                                # chain part A per quadrant, right after
                                # its fb sigmoid: spreads the tanh(c) ACT
                                # ops so PSUM drains are never delayed
                                ctq = ct_sb[q][:, col]
                                c = op_.tile([P, BN], F32, tag="c")
                                tmp = op_.tile([P, BN], F32, tag="tmp")
                                cb = op_.tile([P, BN], F32, tag="cb")
                                # all 4 live until chain B: bufs=4
                                t_ = op_.tile([P, BN], BF16, tag="th",
                                              bufs=4)
                                nc.vector.tensor_mul(c[:], gf[q][:], ctq)
                                nc.vector.tensor_mul(tmp[:], gi[q][:], gz[q][:])
                                nc.vector.tensor_add(c[:], c[:], tmp[:])
                                nc.sync.dma_start(c_d[rows, col], c[:])
                                nc.scalar.activation(t_[:], c[:], AF.Tanh)
                                th[q] = t_
                                nc.vector.tensor_mul(cb[:], gfb[q][:], ctq)
                                nc.vector.tensor_mul(tmp[:], gib[q][:], gz[q][:])
                                nc.vector.tensor_add(cb[:], cb[:], tmp[:])
                                nc.sync.dma_start(cb_d[rows, col], cb[:])
                            else:  # GO: chain part B
                                nc.sync.dma_start(o_d[rows, col], g[:])
                                hh = op_.tile([P, BN], BF16, tag="hh")
                                nc.vector.tensor_mul(hh[:], g[:], th[q][:])
                                nc.sync.dma_start(h_d[rows, col], hh[:])
                    if grp == GD:
                        # softplus for this half: -ln(sigmoid(-wd)),
                        # single Ln + negate over the supertile
                        nc.scalar.activation(sdt[:], sdt[:], AF.Ln)
                        nc.vector.tensor_scalar_mul(sdt[:], sdt[:], -1.0)
                        for q in range(NQ):
                            rows = slice(q * P, (q + 1) * P)
                            nc.sync.dma_start(dr_d[rows, col],
                                              sdt[:, q * BN:(q + 1) * BN])

    nc.compile()
    return nc


def kernel(x, ht, ct, Wx, bx, Wh, bh):
    global _nc_cache, LAST_RESULTS
    if _nc_cache is None:
        _nc_cache = _build()
    nc = _nc_cache

    x = np.ascontiguousarray(x, dtype=np.float32)
    ht = np.ascontiguousarray(ht, dtype=np.float32)
    ct = np.ascontiguousarray(ct, dtype=np.float32)

    # weights: [K2, G] in exec gate order, tiled [28][128][8*128]
    WxT = np.asarray(Wx, dtype=np.float32).T   # [512, 3584]
    WhT = np.asarray(Wh, dtype=np.float32).T
    bsum = np.asarray(bx, dtype=np.float32) + np.asarray(bh, dtype=np.float32)
    w2 = np.empty((K2, G), dtype=NPBF16)
    bbp = np.empty(G, dtype=np.float32)
    for n, old in enumerate(SRC):
        dsl = slice(n * H, (n + 1) * H)
        ssl = slice(old * H, (old + 1) * H)
        w2[:I, dsl] = WxT[:, ssl].astype(NPBF16)
        w2[I:, dsl] = WhT[:, ssl].astype(NPBF16)
        bbp[dsl] = bsum[ssl]
    bbp[0:H] = -bbp[0:H]           # d-gate bias negated (scale=-1 trick)
    # w_stage[grp, p, q*1024 + k*128 + g] = w2[k*128+p, (grp*4+q)*128+g]
    w_stage = np.ascontiguousarray(
        w2.reshape(NK, P, 7, NQ, P).transpose(2, 1, 3, 0, 4)
        .reshape(7, P, NQ * NK * P)
    )
    bbT = np.ascontiguousarray(bbp.reshape(NGT, P).T)   # [128, 28]

    in_maps = []
    for cidx in range(NCORES):
        sl = slice(cidx * BS, (cidx + 1) * BS)
        xh_full = np.empty((K2, BS), dtype=NPBF16)
        xh_full[:I, :] = x[sl].T.astype(NPBF16)
        xh_full[I:, :] = ht[sl].T.astype(NPBF16)
        # [2, 8, 128, 1024] halves-major
        xh_stage = np.ascontiguousarray(
            xh_full.reshape(NK, P, NH, BN).transpose(2, 0, 1, 3)
        )
        ctT = np.ascontiguousarray(ct[sl].T.astype(NPBF16))
        in_maps.append({
            "xh": xh_stage,
            "w": w_stage,
            "ct": ctT,
            "bb": bbT,
        })

    res = run_bass_kernel_spmd(nc, in_maps, core_ids=list(range(NCORES)),
                               trace=TRACE)
    LAST_RESULTS = res

    outs = {}
    for name in ("h", "c", "cb", "o", "dr"):
        full = np.concatenate(
            [res.results[cidx][name] for cidx in range(NCORES)], axis=1
        )
        outs[name] = np.ascontiguousarray(full.T.astype(np.float32))
    return outs["h"], outs["c"], outs["cb"], outs["o"], outs["dr"]
